# revision 31
# baseline (speedup 1.0000x reference)
"""Trainium2 Bass kernel for nn_BinaryMLP (BitNet-ternary SwiGLU MLP).

reference math (fp32):
    s_i = mean(|w_i|)            (per-tensor scalar, i in {1,3,2})
    wq_i = clip(round(w_i/s_i), -1, 1) * s_i     (ternary * scale)
    h1 = x @ w1q.T ; h3 = x @ w3q.T
    y  = (silu(h1) * h3) @ w2q.T

Strategy (8 cores, data-parallel over the 16384 tokens, h-sharded
ternarization):
  - host: pad H 5461->6144 (48 h-tiles, 6 per core), transpose into
    contraction-major layouts (pure layout work, no arithmetic), split
    tokens 8 ways.  Core c's weight slice is h-tiles {c + 8j, j=0..5}
    of all three tensors (columns of w1t/w3t, rows of w2t).
  - device (per core, identical SPMD program):
      warm:    constant-operand matmul spin from t~0 keeps the PE clock
               and HAM duty state warm through the phase-A latency.
      phase A: |w| partial sums of the local slices (DVE abs-reduce,
               slot-0/1 stages kept resident in SBUF for reuse by the
               ternarizer) -> cross-partition sum via DMA transpose
               (keeps the PE stream pure) -> tiny 8-core AllReduce ->
               ternarization thresholds +-s/2.
      phase A2: ternarize local slices only (t2 = Sign(w-s/2)+Sign(w+s/2)
               in {-2,0,+2}, exact in fp8) and AllGather, chunked by slot
               groups (1/2/3 slots) so the first 8 h-tiles of weights
               arrive ~45us after the AllReduce.  w2 gathers last.
      phase B: h-tiles in global order (matches AllGather arrival).
               h1/h3 matmuls vs resident bf16 x (weights stationary, 8
               psum banks), g = silu((s1/2) z1) * ((s3 s2/4) z3) -> bf16
               -> DRAM.  w2 ternary tiles preload into SBUF mid-phase.
      phase C: y[m,d] = sum_h g[h,m] t2[h,d], g stationary, fp32 PSUM.
  - host: concatenate the 8 token shards, reshape to [4, 4096, 2048].

All arithmetic (scales, ternarization, matmuls) happens on device; the
host only reshapes / transposes / pads / slices / concatenates.
"""

import sys
from contextlib import ExitStack

import numpy as np

if "/opt/trn_rl_repo" not in sys.path:
    sys.path.insert(0, "/opt/trn_rl_repo")

import concourse.bass as bass  # noqa: E402,F401
import concourse.mybir as mybir  # noqa: E402
import concourse.tile as tile  # noqa: E402
from concourse import bacc  # noqa: E402

F32 = mybir.dt.float32
BF16 = mybir.dt.bfloat16
FP8 = mybir.dt.float8e4
AF = mybir.ActivationFunctionType
ALU = mybir.AluOpType
AX = mybir.AxisListType

# Full problem geometry (hardcoded per contest rules).
B, S, D = 4, 4096, 2048
H_REAL = 5461
HT = 43                  # real h-tiles (ceil(5461/128))
SLOTS = 6                # h-tiles ternarized per core (48 total, 5 pad)
HP = HT * 128            # 5504: g buffer rows
N_CORES = 8
M = (B * S) // N_CORES   # tokens per core = 2048
KD = D // 128            # 16 k-tiles over D
MC = M // 512            # 4 m-chunks in phase B
N_TRUE = H_REAL * D
AG_GROUPS = ((0,), (1, 2), (3, 4, 5))   # slot groups per w1/w3 AllGather

NWARM = 700              # warm-spin matmuls covering phase-A latency
NWARM2 = 60              # warm burst covering the B->C transition bubble


def build_module(nwarm=NWARM, w13_dt=FP8, w2q_dt=FP8):
    """Build + compile the per-core SPMD Bass module."""
    nc = bacc.Bacc(
        "TRN2",
        target_bir_lowering=False,
        debug=False,
        num_devices=N_CORES,
    )
    xT = nc.dram_tensor("xT", [D, M], F32, kind="ExternalInput").ap()
    wsh1 = nc.dram_tensor("wsh1", [D, SLOTS * 128], F32, kind="ExternalInput").ap()
    wsh3 = nc.dram_tensor("wsh3", [D, SLOTS * 128], F32, kind="ExternalInput").ap()
    wsh2 = nc.dram_tensor("wsh2", [SLOTS * 128, D], F32, kind="ExternalInput").ap()
    y = nc.dram_tensor("y", [M, D], F32, kind="ExternalOutput").ap()

    xview = xT.rearrange("(k p) m -> p k m", p=128)     # [128, KD, M]
    v1 = wsh1.rearrange("(k p) h -> p k h", p=128)      # [128, KD, 768]
    v3 = wsh3.rearrange("(k p) h -> p k h", p=128)
    v2 = wsh2.rearrange("(s p) d -> p s d", p=128)      # [128, SLOTS, D]

    with tile.TileContext(nc) as tc:
        with ExitStack() as ctx:
            dram = ctx.enter_context(tc.tile_pool(name="dram", bufs=1, space="DRAM"))
            g_dram = dram.tile([HP, M], BF16, tag="g", name="g")
            cc_in = dram.tile([1, 8], F32, tag="cc_in", name="cc_in")
            cc_out = dram.tile([1, 8], F32, tag="cc_out", name="cc_out")
            tp8 = dram.tile([8, 128], F32, tag="tp8", name="tp8")
            g_rd = g_dram.rearrange("(k p) m -> p k m", p=128)

            # k-contig row layout [p, (k t h)]: gathered rank blocks are 128
            # fully-contiguous rows -> single fat packet per partition on load
            agin13 = []
            agout13 = []
            for q, sl in enumerate(AG_GROUPS):
                w = KD * 2 * len(sl) * 128
                agin13.append(dram.tile(
                    [128, w], w13_dt, tag=f"agi{q}", name=f"agi{q}"))
                agout13.append(dram.tile(
                    [N_CORES * 128, w], w13_dt, tag=f"ago{q}", name=f"ago{q}",
                    addr_space="Shared"))
            agin2 = dram.tile([SLOTS * 128, D], w2q_dt, tag="agi2", name="agi2")
            agout2 = dram.tile([N_CORES * SLOTS * 128, D], w2q_dt,
                               tag="ago2", name="ago2", addr_space="Shared")
            agout2v = agout2.rearrange("(r s p) d -> p r s d", r=N_CORES, p=128)

            pc = ctx.enter_context(tc.tile_pool(name="pconst", bufs=1))
            bias = {}
            w2qk = [None] * HT

            with ExitStack() as big:
                zps = big.enter_context(
                    tc.tile_pool(name="zps", bufs=8, space="PSUM"))
                xp = big.enter_context(tc.tile_pool(name="xp", bufs=1))

                # ---- warm: constant-operand spin, first on the PE queue
                warm_a = xp.tile([128, 128], BF16, tag="warm_a", name="warm_a")
                warm_b = xp.tile([128, 512], BF16, tag="warm_b", name="warm_b")
                nc.vector.memset(warm_a, 0.03125)
                nc.vector.memset(warm_b, 0.03125)
                wz = zps.tile([128, 512], F32, tag="z", name="z")
                for i in range(nwarm):
                    nc.tensor.matmul(wz, lhsT=warm_a, rhs=warm_b,
                                     start=(i == 0), stop=(i == nwarm - 1))

                x_sb = xp.tile([128, KD, M], BF16, tag="x_sb", name="x_sb")
                xstg = []
                with ExitStack() as sx:
                    xstg_p = sx.enter_context(tc.tile_pool(name="xstg", bufs=2))

                    # ------------- phase A: scales ----------------------
                    # fat contiguous stages: w1/w3 in 2 chunks of 3 slots,
                    # w2 in 3 chunks of 2 slots (1.5-4KB bursts)
                    with tc.tile_pool(name="keep01", bufs=1) as keep_p, \
                         tc.tile_pool(name="scstg", bufs=2) as sc_pool, \
                         tc.tile_pool(name="scsum", bufs=1) as sm_pool:
                        # resident slot-0 stage of w1/w3 (reused by tern)
                        k13 = keep_p.tile([128, 2, KD, 128], F32,
                                          tag="k13", name="k13")
                        for t, vt in ((0, v1), (1, v3)):
                            nc.sync.dma_start(k13[:, t], vt[:, :, 0:128])
                        asum = sm_pool.tile([128, 18], F32, tag="asum", name="asum")
                        nc.vector.memset(asum, 0.0)
                        for t, vt in ((0, v1), (1, v3)):
                            for ci in range(3):
                                stg = sc_pool.tile([128, 4096], F32,
                                                   tag="scstg", name="scstg")
                                nc.sync.dma_start(
                                    stg.rearrange("p (a h) -> p a h", h=256),
                                    vt[:, :, ci * 256:(ci + 1) * 256])
                                nc.vector.tensor_reduce(
                                    asum[:, t * 6 + ci:t * 6 + ci + 1], stg,
                                    axis=AX.X, op=ALU.add,
                                    apply_absolute_value=True)
                        for ci in range(3):
                            stg = sc_pool.tile([128, 4096], F32,
                                               tag="scstg", name="scstg")
                            nc.sync.dma_start(
                                stg.rearrange("p (s d) -> p s d", d=D),
                                v2[:, ci * 2:(ci + 1) * 2, :])
                            nc.vector.tensor_reduce(
                                asum[:, 12 + ci:13 + ci], stg,
                                axis=AX.X, op=ALU.add,
                                apply_absolute_value=True)
                        # x stages follow the scale reads on the sync queue
                        for k in range(KD):
                            st = xstg_p.tile([128, M], F32, tag="xstg",
                                             name="xstg")
                            nc.sync.dma_start(st, xview[:, k, :])
                            xstg.append(st)
                        part8 = sm_pool.tile([128, 8], F32, tag="part8",
                                             name="part8")
                        nc.vector.memset(part8, 0.0)
                        for t in range(3):
                            nc.vector.tensor_reduce(
                                part8[:, t:t + 1], asum[:, t * 6:(t + 1) * 6],
                                axis=AX.X, op=ALU.add)
                        # cross-partition sum via DMA transpose (no PE);
                        # scalar queue so the big sync-queue stages don't
                        # delay the AllReduce input
                        nc.scalar.dma_start(tp8.rearrange("q p -> p q"), part8)
                        tp_sb = sm_pool.tile([8, 128], F32, tag="tp_sb",
                                             name="tp_sb")
                        nc.scalar.dma_start(tp_sb, tp8)
                        ssum8 = sm_pool.tile([8, 1], F32, tag="ssum8",
                                             name="ssum8")
                        nc.vector.tensor_reduce(ssum8, tp_sb, axis=AX.X,
                                                op=ALU.add)
                        nc.scalar.dma_start(cc_in.rearrange("a b -> b a"),
                                            ssum8)
                        nc.gpsimd.collective_compute(
                            "AllReduce", ALU.add,
                            replica_groups=[list(range(N_CORES))],
                            ins=[cc_in.opt()], outs=[cc_out.opt()],
                        )
                        g8 = sm_pool.tile([1, 8], F32, tag="g8", name="g8")
                        nc.scalar.dma_start(g8, cc_out)
                        gb = pc.tile([128, 8], F32, tag="gb", name="gb")
                        nc.gpsimd.partition_broadcast(gb, g8)
                        for t, name in enumerate(["w1", "w3", "w2"]):
                            for sgn in ("p", "n"):
                                bias[name + sgn] = pc.tile(
                                    [128, 1], F32, tag=f"b_{name}{sgn}",
                                    name=f"b_{name}{sgn}")
                                k = 0.5 / N_TRUE if sgn == "p" else -0.5 / N_TRUE
                                nc.vector.tensor_scalar(
                                    bias[name + sgn], gb[:, t:t + 1], k, None,
                                    ALU.mult)
                        s23 = pc.tile([128, 1], F32, tag="s23", name="s23")
                        nc.vector.tensor_mul(s23, bias["w3p"], bias["w2p"])

                        # ------- phase A2: ternarize local + AllGather ----
                        with tc.tile_pool(name="qstg", bufs=2) as qstg_p, \
                             tc.tile_pool(name="qq", bufs=2) as qq_p:

                            def quantize(out_ap, stg_ap, bn, bp, dt, eng):
                                fw = stg_ap.shape[-1]
                                if eng == "act":
                                    qa = qq_p.tile([128, 2048], dt, tag="qa",
                                                   name="qa")
                                    qb = qq_p.tile([128, 2048], dt, tag="qb",
                                                   name="qb")
                                    nc.scalar.activation(
                                        qa[:, :fw], stg_ap, AF.Sign, bias=bn)
                                    nc.scalar.activation(
                                        qb[:, :fw], stg_ap, AF.Sign, bias=bp)
                                    nc.vector.tensor_add(
                                        out_ap, qa[:, :fw], qb[:, :fw])
                                else:
                                    qa = qq_p.tile([128, 2048], dt, tag="da",
                                                   name="da")
                                    qb = qq_p.tile([128, 2048], dt, tag="db",
                                                   name="db")
                                    nc.vector.tensor_scalar(
                                        qa[:, :fw], stg_ap, bp, 2.0,
                                        ALU.is_ge, ALU.mult)
                                    nc.vector.tensor_scalar(
                                        qb[:, :fw], stg_ap, bn, 2.0,
                                        ALU.is_lt, ALU.mult)
                                    nc.vector.tensor_sub(
                                        out_ap, qa[:, :fw], qb[:, :fw])

                            for q, sl in enumerate(AG_GROUPS):
                                aginv = agin13[q].rearrange(
                                    "p (k two h) -> p k two h",
                                    two=2, h=len(sl) * 128)
                                for s in sl:
                                    off = s - sl[0]
                                    for t, vt in ((0, v1), (1, v3)):
                                        if s < 1:
                                            src = k13[:, t].rearrange(
                                                "p a h -> p (a h)")
                                        else:
                                            stg = qstg_p.tile(
                                                [128, KD * 128], F32,
                                                tag="qstg", name="qstg")
                                            nc.sync.dma_start(
                                                stg.rearrange(
                                                    "p (a h) -> p a h", h=128),
                                                vt[:, :, s * 128:(s + 1) * 128])
                                            src = stg
                                        qt = qq_p.tile([128, KD * 128], w13_dt,
                                                       tag="qt", name="qt")
                                        bn = bias["w1n" if t == 0 else "w3n"]
                                        bp = bias["w1p" if t == 0 else "w3p"]
                                        # slot 0: both tensors on ACT so the
                                        # first AllGather gates only on ACT
                                        eng = "act" if (s == 0 or t == 0) \
                                            else "dve"
                                        quantize(qt, src, bn, bp, w13_dt, eng)
                                        nc.scalar.dma_start(
                                            aginv[:, :, t,
                                                  off * 128:(off + 1) * 128],
                                            qt.rearrange(
                                                "p (a h) -> p a h", h=128))
                                nc.gpsimd.collective_compute(
                                    "AllGather", ALU.bypass,
                                    replica_groups=[list(range(N_CORES))],
                                    ins=[agin13[q].opt()],
                                    outs=[agout13[q].opt()],
                                )

                            agin2v = agin2.rearrange("(s p) d -> p s d", p=128)
                            for s in range(SLOTS):
                                stg = qstg_p.tile([128, KD * 128], F32,
                                                  tag="qstg", name="qstg")
                                nc.sync.dma_start(stg, v2[:, s, :])
                                qt = qq_p.tile([128, KD * 128], w2q_dt,
                                               tag="qt", name="qt")
                                quantize(qt, stg, bias["w2n"], bias["w2p"],
                                         w2q_dt, "act" if s % 2 == 0 else "dve")
                                nc.scalar.dma_start(agin2v[:, s, :], qt)
                            nc.gpsimd.collective_compute(
                                "AllGather", ALU.bypass,
                                replica_groups=[list(range(N_CORES))],
                                ins=[agin2.opt()],
                                outs=[agout2.opt()],
                            )

                    # x cast to resident bf16 (DVE; casts run post-stage)
                    for k in range(KD):
                        nc.vector.tensor_copy(x_sb[:, k, :], xstg[k])

                # ---------------- phase B -------------------------------
                wq_p = big.enter_context(tc.tile_pool(name="wq", bufs=3))
                sl_p = big.enter_context(tc.tile_pool(name="slp", bufs=3))
                g_p = big.enter_context(tc.tile_pool(name="gp", bufs=2))
                # right side: outlives `big` without breaking LIFO order
                w2c_p = ctx.enter_context(
                    tc.tile_pool(name="w2c", bufs=HT, side="right"))
                ntile = 0
                for q, sl in enumerate(AG_GROUPS):
                    ns = len(sl)
                    for r in range(N_CORES):
                        tiles = [(s, r + N_CORES * s) for s in sl
                                 if r + N_CORES * s < HT]
                        if not tiles:
                            continue
                        # one fat DMA per (group, rank): 128 contiguous rows,
                        # triggered from the idle sync queue so prefetch is
                        # not serialized behind ACT compute
                        wqf = wq_p.tile([128, KD * 2 * 3 * 128], w13_dt,
                                        tag="wq", name="wq")
                        nc.sync.dma_start(
                            wqf[:, :KD * 2 * ns * 128],
                            agout13[q][r * 128:(r + 1) * 128, :])
                        wq = wqf[:, :KD * 2 * ns * 128].rearrange(
                            "p (k two h) -> p k two h", two=2, h=ns * 128)
                        for s, gt in tiles:
                            hs = slice((s - sl[0]) * 128, (s - sl[0] + 1) * 128)
                            zz = [[None] * MC, [None] * MC]
                            for t in range(2):
                                for mci in range(MC):
                                    zz[t][mci] = zps.tile([128, 512], F32,
                                                          tag="z", name="z")
                                for k in range(KD):
                                    for mci in range(MC):
                                        nc.tensor.matmul(
                                            zz[t][mci], lhsT=wq[:, k, t, hs],
                                            rhs=x_sb[:, k,
                                                     mci * 512:(mci + 1) * 512],
                                            start=(k == 0), stop=(k == KD - 1),
                                        )
                            g_t = g_p.tile([128, M], BF16, tag="g_t",
                                           name="g_t")
                            for mci in range(MC):
                                ms = slice(mci * 512, (mci + 1) * 512)
                                sl_ = sl_p.tile([128, 512], BF16, tag="sl",
                                                name="sl")
                                nc.scalar.activation(sl_, zz[0][mci], AF.Silu,
                                                     bias=0.0,
                                                     scale=bias["w1p"])
                                sc_ = sl_p.tile([128, 512], BF16, tag="sc",
                                                name="sc")
                                nc.scalar.activation(sc_, zz[1][mci], AF.Copy,
                                                     bias=0.0, scale=s23)
                                nc.vector.tensor_mul(g_t[:, ms], sl_, sc_)
                            nc.scalar.dma_start(
                                g_dram[gt * 128:(gt + 1) * 128, :], g_t)
                            ntile += 1
                            if ntile == 28:
                                # preload ternary w2 into SBUF (AllGather
                                # long done; ~15 tiles of DMA slack left)
                                for t2 in range(HT):
                                    w2qk[t2] = w2c_p.tile(
                                        [128, D], w2q_dt, tag="w2qk",
                                        name="w2qk")
                                    nc.scalar.dma_start(
                                        w2qk[t2],
                                        agout2v[:, t2 % N_CORES,
                                                t2 // N_CORES, :])

                # warm burst: keeps PE duty state hot through the
                # g-store -> gq-load bubble at the B->C transition
                wz2 = zps.tile([128, 512], F32, tag="z", name="z")
                for i in range(NWARM2):
                    nc.tensor.matmul(wz2, lhsT=warm_a, rhs=warm_b,
                                     start=(i == 0), stop=(i == NWARM2 - 1))

            # ---------------- phase C -----------------------------------
            ndc = D // 512
            with ExitStack() as scx:
                gq_p = scx.enter_context(tc.tile_pool(name="gq", bufs=3))
                y_p = scx.enter_context(tc.tile_pool(name="yp", bufs=2 * ndc))
                yps = scx.enter_context(
                    tc.tile_pool(name="yps", bufs=8, space="PSUM"))
                # first load is 1 m-tile (lower latency out of phase B),
                # then 2 m-tiles per load (512B bursts)
                groups = [[0]] + [[1 + 2 * i, 2 + 2 * i] for i in range(7)] \
                    + [[15]]
                for mts in groups:
                    nm = len(mts)
                    gq = gq_p.tile([128, HT, 256], BF16, tag="gq", name="gq")
                    nc.sync.dma_start(
                        gq[:, :, :nm * 128],
                        g_rd[:, :, mts[0] * 128:(mts[0] + nm) * 128])
                    for mi, mt in enumerate(mts):
                        yp4 = [yps.tile([128, 512], F32, tag="yps", name="yps")
                               for _ in range(ndc)]
                        for k2 in range(HT):
                            for di in range(ndc):
                                nc.tensor.matmul(
                                    yp4[di],
                                    lhsT=gq[:, k2, mi * 128:(mi + 1) * 128],
                                    rhs=w2qk[k2][:, di * 512:(di + 1) * 512],
                                    start=(k2 == 0), stop=(k2 == HT - 1),
                                )
                        for di in range(ndc):
                            ysb = y_p.tile([128, 512], F32, tag="ysb",
                                           name="ysb")
                            # split eviction across ACT and DVE
                            if di % 2 == 0:
                                nc.scalar.copy(ysb, yp4[di])
                            else:
                                nc.vector.tensor_copy(ysb, yp4[di])
                            nc.scalar.dma_start(
                                y[mt * 128:(mt + 1) * 128,
                                  di * 512:(di + 1) * 512],
                                ysb)

    nc.compile()
    return nc


_NC_CACHE = {}


def _get_module():
    if "nc" not in _NC_CACHE:
        _NC_CACHE["nc"] = build_module()
    return _NC_CACHE["nc"]


def prep_inputs(x, w1, w3, w2):
    """Host-side layout work: pad, transpose, shard, slice. No arithmetic."""
    h_real = w1.shape[0]
    hp48 = N_CORES * SLOTS * 128          # 6144
    x = np.ascontiguousarray(np.asarray(x, dtype=np.float32))
    xf = x.reshape(-1, D)
    w1t = np.zeros((D, hp48), np.float32)
    w1t[:, :h_real] = np.asarray(w1, np.float32).T
    w3t = np.zeros((D, hp48), np.float32)
    w3t[:, :h_real] = np.asarray(w3, np.float32).T
    w2t = np.zeros((hp48, D), np.float32)
    w2t[:h_real, :] = np.asarray(w2, np.float32).T

    in_maps = []
    for c in range(N_CORES):
        xc = np.ascontiguousarray(xf[c * M:(c + 1) * M].T)   # [D, M]
        # core c ternarizes h-tiles {c + 8j}
        cols = np.concatenate([
            np.arange((c + N_CORES * j) * 128, (c + N_CORES * j) * 128 + 128)
            for j in range(SLOTS)
        ])
        in_maps.append({
            "xT": xc,
            "wsh1": np.ascontiguousarray(w1t[:, cols]),
            "wsh3": np.ascontiguousarray(w3t[:, cols]),
            "wsh2": np.ascontiguousarray(w2t[cols, :]),
        })
    return in_maps


def kernel(x, w1, w3, w2):
    from concourse.bass_utils import run_bass_kernel_spmd

    nc = _get_module()
    in_maps = prep_inputs(x, w1, w3, w2)
    res = run_bass_kernel_spmd(nc, in_maps, core_ids=list(range(N_CORES)))
    _NC_CACHE["last_results"] = res
    yf = np.concatenate([r["y"] for r in res.results], axis=0)  # [16384, 2048]
    return np.ascontiguousarray(yf.reshape(B, S, D).astype(np.float32))
